# revision 8
# baseline (speedup 1.0000x reference)
"""Trainium2 Bass kernel for nn_BilinearChebConv (bilinear Chebyshev graph conv).

out[o] = sum_{i,j} theta[i,j,0,o] * T_i(Lr) @ x @ T_j(Lc) + bias[o]

Structure (per core c, rows m_c = c*192 .. c*192+191):
    B_i = T_i(Lr)[:, m_c]            (thin column-slice Chebyshev recursion)
    W_0 = x^T @ [B_0 .. B_4]         (n on partitions, (i, m) stacked on free)
    W_j = 2 Lc W_{j-1} - W_{j-2}     (all 5 i-chains in one recursion)
    out[o, m_c, n] = sum_ij theta[ijo] W_j[n, (i, m)] + bias[o]

v3 changes vs v2:
  - theta stage quarters cover contiguous 384-column n-ranges (quarter q
    owns wd blocks 3q..3q+2), so PSUM partition (q,o) rows map to runs of
    contiguous output columns; the device emits out[OUT, 192, 1536]
    directly and host reassembly is a big-block slice assign (~50ms
    instead of a 1.6s 6-D gather).
  - Lr/Lc uploaded unscaled; the Chebyshev 2x lives in the recursion's
    scalar_tensor_tensor scalar. b01 replaced by a static identity-slice
    input (built once at import) + a thin Lr column slice; negi removed.
  - host prep uses persistent pre-concatenated buffers and assignment
    casts (numpy assignment cast is vectorized, .astype is not).
  - output device buffers are recycled across calls (donated), so the
    151MB zeros upload happens only on the first call.
"""

import sys

sys.path.insert(0, "/opt/trn_rl_repo")

import numpy as np
import ml_dtypes

M = 1536
N = 1536
NCORES = 8
ML = M // NCORES          # 192 rows per core
MT = 64                   # max m-stage width
MTS = [64, 64, 64]
OFFS = [0, 64, 128]
NST = len(MTS)
OUT = 32
KB = M // 128             # 12 partition blocks
P = 128
BP = ML                   # per-i column block in B (192, no padding in bf16)

_BF = ml_dtypes.bfloat16
_BUILT = None


def _build_program():
    import concourse.bacc as bacc
    import concourse.mybir as mybir
    from concourse import tile

    F32 = mybir.dt.float32
    BF16 = mybir.dt.bfloat16

    nc = bacc.Bacc(num_devices=NCORES)

    x_d = nc.dram_tensor("x", [M, N], BF16, kind="ExternalInput")
    lr_d = nc.dram_tensor("lr", [M, M], BF16, kind="ExternalInput")
    lc_d = nc.dram_tensor("lc", [N, N], BF16, kind="ExternalInput")
    # ec: per-core identity column slice E_c = I[:, m_c] (static)
    ec_d = nc.dram_tensor("ec", [M, BP], BF16, kind="ExternalInput")
    # lrc1: Lr[:, m_c] (B_1 of the row recursion)
    lrc1_d = nc.dram_tensor("lrc1", [M, BP], BF16, kind="ExternalInput")
    # host-transposed x rows for this core: x^T[:, m_c]  (W0's i=0 segment)
    xct_d = nc.dram_tensor("xct", [N, ML], BF16, kind="ExternalInput")
    thf_d = nc.dram_tensor("thetaf", [100, 4 * OUT], BF16, kind="ExternalInput")
    bias_d = nc.dram_tensor("biasr", [P, 1], F32, kind="ExternalInput")
    # W_{0..4} spill per stage: [nb][j][i][n][m] — (j,i) adjacent and (n,m)
    # contiguous per (j,i), so each theta gather is ONE fat 25-partition DMA
    wd_s = [
        nc.dram_tensor(f"wds{s}", [KB, 5, 5, P, MTS[s]], BF16, kind="Internal")
        for s in range(NST)
    ]
    # direct per-core output layout; host does out[:, m_c, :] = outc[c]
    out_d = nc.dram_tensor("outc", [OUT, ML, N], BF16, kind="ExternalOutput")

    with tile.TileContext(nc) as tc:
        with tc.tile_pool(name="const", bufs=1) as constp:
            thsb = constp.tile([100, 4 * OUT], BF16, tag="thf")
            nc.sync.dma_start(thsb[:], thf_d[:])
            biast = constp.tile([P, 1], F32, tag="bias")
            nc.sync.dma_start(biast[:], bias_d[:])

            # w0res persists from Phase R into Phase C
            with tc.tile_pool(name="w0res", bufs=1) as w0p:
                w0res = []
                for nb in range(KB):
                    w0res.append(
                        w0p.tile([P, 5 * ML], BF16, tag=f"w0_{nb}", name=f"w0res{nb}")
                    )

                # ---------------- Phase R: row stage ----------------
                with (
                    tc.tile_pool(name="lrp", bufs=1) as lrp,
                    tc.tile_pool(name="bpad", bufs=1) as bpadp,
                    tc.tile_pool(name="xs", bufs=1) as xp,
                    tc.tile_pool(name="brps", bufs=2, space="PSUM") as brps,
                    tc.tile_pool(name="w0ps", bufs=2, space="PSUM") as w0ps,
                ):
                    bt = []
                    for k in range(KB):
                        t_ = bpadp.tile([P, 5 * BP], BF16, tag=f"bp{k}", name=f"bt{k}")
                        [nc.scalar, nc.sync][k % 2].dma_start(
                            t_[:, 0:BP], ec_d[k * P : (k + 1) * P, :]
                        )
                        [nc.sync, nc.scalar][k % 2].dma_start(
                            t_[:, BP : 2 * BP], lrc1_d[k * P : (k + 1) * P, :]
                        )
                        bt.append(t_)
                    xt = []
                    for kp in range(KB // 2):
                        t_ = xp.tile([P, 2 * N], BF16, tag=f"x{kp}", name=f"xt{kp}")
                        dst = t_.rearrange("p (k n) -> p k n", k=2)
                        src = x_d[2 * kp * P : (2 * kp + 2) * P, :].rearrange(
                            "(k p) n -> p k n", k=2
                        )
                        nc.gpsimd.dma_start(dst, src)
                        xt.append(t_[:, 0:N])
                        xt.append(t_[:, N : 2 * N])
                    lrt = []
                    for kp in range(KB // 2):
                        t_ = lrp.tile([P, 2 * M], BF16, tag=f"lr{kp}", name=f"lrt{kp}")
                        dst = t_.rearrange("p (k n) -> p k n", k=2)
                        src = lr_d[2 * kp * P : (2 * kp + 2) * P, :].rearrange(
                            "(k p) n -> p k n", k=2
                        )
                        [nc.sync, nc.scalar][kp % 2].dma_start(dst, src)
                        lrt.append(t_[:, 0:M])
                        lrt.append(t_[:, M : 2 * M])

                    import concourse.mybir as mybir
                    SUB0 = mybir.AluOpType.subtract
                    MULT0 = mybir.AluOpType.mult

                    # W0 segment A: i = 0 comes straight from the host-side
                    # transpose of x's m_c rows; i = 1 (cols 192:384) is the
                    # only matmul here — runs while lr loads / B-rec warm up
                    for nb in range(KB):
                        [nc.sync, nc.scalar][nb % 2].dma_start(
                            w0res[nb][:, 0:BP], xct_d[nb * P : (nb + 1) * P, :]
                        )
                        psA = w0ps.tile([P, BP], F32, tag="w0sA", name="w0psA")
                        for k in range(KB):
                            nc.tensor.matmul(
                                psA[:],
                                lhsT=xt[k][:, nb * P : (nb + 1) * P],
                                rhs=bt[k][:, BP : 2 * BP],
                                start=(k == 0),
                                stop=(k == KB - 1),
                            )
                        nc.vector.tensor_copy(w0res[nb][:, BP : 2 * BP], psA[:])

                    # B recursion: B_i = 2 Lr @ B_{i-1} - B_{i-2}, i = 2..4
                    for i in range(2, 5):
                        for p in range(KB):
                            ps = brps.tile([P, BP], F32, tag="brec", name="brps")
                            for k in range(KB):
                                nc.tensor.matmul(
                                    ps[:],
                                    lhsT=lrt[k][:, p * P : (p + 1) * P],
                                    rhs=bt[k][:, (i - 1) * BP : i * BP],
                                    start=(k == 0),
                                    stop=(k == KB - 1),
                                )
                            nc.vector.scalar_tensor_tensor(
                                bt[p][:, i * BP : (i + 1) * BP],
                                ps[:],
                                2.0,
                                bt[p][:, (i - 2) * BP : (i - 1) * BP],
                                MULT0,
                                SUB0,
                            )

                    # W0 segment B: i = 2..4 (cols 384:960), after B-rec
                    segs = [(2 * BP, 512), (2 * BP + 512, 3 * BP - 512)]
                    for nb in range(KB):
                        pss = [
                            w0ps.tile([P, sz], F32, tag=f"w0s{si}", name=f"w0ps{si}")
                            for si, (_, sz) in enumerate(segs)
                        ]
                        for k in range(KB):
                            for ps, (off, sz) in zip(pss, segs):
                                nc.tensor.matmul(
                                    ps[:],
                                    lhsT=xt[k][:, nb * P : (nb + 1) * P],
                                    rhs=bt[k][:, off : off + sz],
                                    start=(k == 0),
                                    stop=(k == KB - 1),
                                )
                        for ps, (off, sz) in zip(pss, segs):
                            nc.vector.tensor_copy(w0res[nb][:, off : off + sz], ps[:])
                        w0v3 = w0res[nb].rearrange("n (i m) -> n i m", i=5)
                        for si in range(NST):
                            [nc.sync, nc.scalar, nc.gpsimd][(nb + si) % 3].dma_start(
                                wd_s[si][nb, 0].rearrange("i n m -> n i m"),
                                w0v3[:, :, OFFS[si] : OFFS[si] + MTS[si]],
                            )

                # ---------------- Phase C: column stage + theta ----------------
                with (
                    tc.tile_pool(name="lcp", bufs=1) as lcp,
                    tc.tile_pool(name="wp", bufs=4) as wp,
                    tc.tile_pool(name="zfp", bufs=8) as zfp,
                    tc.tile_pool(name="evp", bufs=1) as evp,
                    tc.tile_pool(name="wps", bufs=2, space="PSUM") as wps,
                    tc.tile_pool(name="thps", bufs=1, space="PSUM") as thps,
                ):
                    lct = []
                    for k in range(KB):
                        t_ = lcp.tile([P, N], BF16, tag=f"lc{k}", name=f"lct{k}")
                        [nc.sync, nc.scalar, nc.gpsimd][k % 3].dma_start(
                            t_[:], lc_d[k * P : (k + 1) * P, :]
                        )
                        lct.append(t_)

                    import concourse.mybir as mybir
                    SUB = mybir.AluOpType.subtract
                    MULT = mybir.AluOpType.mult

                    # j=4 chunk order: complete quarter-groups {u,3+u,6+u,9+u}
                    # early so theta steps can interleave
                    J4ORDER = [0, 3, 6, 9, 1, 4, 7, 10, 2, 5, 8, 11]

                    def jrec_chunks(s):
                        """Yield (j, nb, emit_fn) for stage s's j-recursion."""
                        mt, off = MTS[s], OFFS[s]
                        wcur = [[None] * KB for _ in range(5)]
                        w0v = [
                            w0res[k]
                            .rearrange("n (i m) -> n i m", i=5)[:, :, off : off + mt]
                            for k in range(KB)
                        ]

                        for j in range(1, 5):
                            nbs = J4ORDER if j == 4 else range(KB)
                            for nb in nbs:
                                def chunk(j=j, nb=nb):
                                    psf = wps.tile(
                                        [P, 5 * MT], F32, tag="wrec", name=f"wps{j}"
                                    )
                                    ps = psf[:, 0 : 5 * mt]
                                    rhs_prev = (
                                        w0v
                                        if j == 1
                                        else [w[:, 0 : 5 * mt] for w in wcur[j - 1]]
                                    )
                                    for k in range(KB):
                                        nc.tensor.matmul(
                                            ps,
                                            lhsT=lct[k][:, nb * P : (nb + 1) * P],
                                            rhs=rhs_prev[k],
                                            start=(k == 0),
                                            stop=(k == KB - 1),
                                        )
                                    w = wp.tile(
                                        [P, 5 * MT], BF16, tag=f"w_{nb}",
                                        name=f"w{j}t{nb}",
                                    )
                                    if j == 1:
                                        nc.vector.tensor_copy(w[:, 0 : 5 * mt], ps)
                                    else:
                                        prev2 = (
                                            w0v[nb]
                                            if j == 2
                                            else wcur[j - 2][nb][
                                                :, 0 : 5 * mt
                                            ].rearrange("n (i m) -> n i m", i=5)
                                        )
                                        nc.vector.scalar_tensor_tensor(
                                            w[:, 0 : 5 * mt].rearrange(
                                                "n (i m) -> n i m", i=5
                                            ),
                                            ps.rearrange("n (i m) -> n i m", i=5),
                                            2.0,
                                            prev2,
                                            MULT,
                                            SUB,
                                        )
                                    wcur[j][nb] = w
                                    eng = nc.sync if nb % 2 == 0 else nc.scalar
                                    eng.dma_start(
                                        wd_s[s][nb, j].rearrange("i n m -> n i m"),
                                        w[:, 0 : 5 * mt],
                                    )
                                yield j, nb, chunk

                    engs = [nc.gpsimd, nc.sync, nc.scalar]

                    def gather(s, t, zft):
                        # theta step t covers n = q*384 + t*32 .. +32 per
                        # quarter q, i.e. wd block 3q + t//4, cols (t%4)*32
                        mt = MTS[s]
                        u, v = t // 4, t % 4
                        zff = zfp.tile([100, 32 * MT], BF16, tag="zf", name=f"zf{t%8}")
                        zf = zff[:, 0 : 32 * mt]
                        for q in range(4):
                            src = wd_s[s][3 * q + u].rearrange(
                                "j i n m -> (j i) n m"
                            )
                            zv = zf.rearrange("p (n m) -> p n m", n=32)[
                                q * 25 : (q + 1) * 25
                            ]
                            engs[(4 * t + q) % 3].dma_start(
                                zv, src[:, v * 32 : (v + 1) * 32, :]
                            )
                        zft[t] = zff

                    def theta_compute(s, t, ev, zft):
                        mt = MTS[s]
                        # ev free layout (m mt, t KB, nq 32)
                        evv = ev.rearrange("p (ml t n) -> p ml t n", t=KB, n=32)
                        zff = zft.pop(t)
                        zm = zff[:, 0 : 32 * mt].rearrange("p (n m) -> p m n", n=32)
                        for kk2 in range(mt // 32):
                            ps = thps.tile(
                                [P, 1024], F32, tag=f"th{kk2%2}", name=f"thps{kk2}"
                            )
                            for h in range(2):
                                kk = kk2 * 2 + h
                                nc.tensor.matmul(
                                    ps[:, h * 512 : (h + 1) * 512],
                                    lhsT=thsb[:],
                                    rhs=zm[:, kk * 16 : (kk + 1) * 16, :],
                                    start=True,
                                    stop=True,
                                )
                            dst = evv[:, kk2 * 32 : (kk2 + 1) * 32, t, :]
                            srcp = ps.rearrange("p (m n) -> p m n", m=32)
                            nc.vector.tensor_scalar_add(dst, srcp, biast[:])

                    def out_dmas(s, ev, piece, npieces):
                        # partition (q,o) holds n = q*384 + t*32 + nq —
                        # contiguous runs in the [OUT, ML, N] output; one
                        # DMA per quarter (32-partition SBUF slice)
                        mt = MTS[s]
                        tpp = KB // npieces
                        t0, t1 = piece * tpp, (piece + 1) * tpp
                        evq = ev.rearrange("p (ml t n) -> p ml t n", t=KB, n=32)
                        for q in range(4):
                            dst = out_d[
                                :,
                                OFFS[s] : OFFS[s] + mt,
                                q * 384 : (q + 1) * 384,
                            ].rearrange("o m (t n) -> o m t n", n=32)[:, :, t0:t1, :]
                            src = evq[q * 32 : (q + 1) * 32, 0:mt, t0:t1, :]
                            [nc.scalar, nc.sync, nc.gpsimd][
                                (4 * piece + q) % 3
                            ].dma_start(dst, src)

                    # pipeline: quarter-group u's gathers fire once wd blocks
                    # {u, 3+u, 6+u, 9+u} are spilled; theta steps trail them
                    for s in range(NST):
                        last = s == NST - 1
                        npieces = 3 if last else 1
                        ev = evp.tile([P, MT * KB * 32], BF16, tag="ev", name=f"ev{s}")
                        zft = {}
                        done = [0]

                        def step(s=s, ev=ev, zft=zft, done=done, npieces=npieces):
                            t = done[0]
                            done[0] += 1
                            theta_compute(s, t, ev, zft)
                            d = done[0]
                            if d % (KB // npieces) == 0 and d < KB:
                                out_dmas(s, ev, d // (KB // npieces) - 1, npieces)

                        k = 0
                        for j, nb, chunk in jrec_chunks(s):
                            chunk()
                            if j == 4:
                                k += 1
                                if k % 4 == 0:
                                    u = k // 4 - 1
                                    for t in range(4 * u, 4 * u + 4):
                                        gather(s, t, zft)
                                if k > 4 and done[0] < 4 * (k // 4):
                                    step()
                        while done[0] < KB:
                            step()
                        out_dmas(s, ev, npieces - 1, npieces)

    nc.finalize()
    return nc


_HOST = None


def _host_state():
    """Persistent pre-concatenated input buffers (built once)."""
    global _HOST
    if _HOST is not None:
        return _HOST
    bufs = {
        "x": np.empty((NCORES, M, N), _BF),
        "lr": np.empty((NCORES, M, M), _BF),
        "lc": np.empty((NCORES, N, N), _BF),
        "ec": np.zeros((NCORES, M, BP), _BF),
        "lrc1": np.empty((NCORES, M, BP), _BF),
        "xct": np.empty((NCORES, N, ML), _BF),
        "thetaf": np.zeros((NCORES, 100, 4 * OUT), _BF),
        "biasr": np.empty((NCORES, P, 1), np.float32),
    }
    one = np.ones((ML,), np.float32)
    for c in range(NCORES):
        bufs["ec"][c, c * ML : (c + 1) * ML, :][np.arange(ML), np.arange(ML)] = one
    _HOST = bufs
    return bufs


def _fill_inputs(x, Lr, Lc, theta, bias):
    b = _host_state()
    x2 = x.reshape(M, N)
    b["x"][...] = x2[None]
    b["lr"][...] = Lr[None]
    b["lc"][...] = Lc[None]
    xT = x2.T
    for c in range(NCORES):
        cols = slice(c * ML, (c + 1) * ML)
        b["lrc1"][c] = Lr[:, cols]
        b["xct"][c] = xT[:, cols]
    thf25 = np.zeros((25, OUT), np.float32)
    th = theta.reshape(5, 5, OUT)
    for i in range(5):
        for j in range(5):
            thf25[j * 5 + i] = th[i, j]
    for q in range(4):
        b["thetaf"][:, q * 25 : (q + 1) * 25, q * OUT : (q + 1) * OUT] = thf25[None]
    b["biasr"][...] = np.tile(bias.astype(np.float32), 4).reshape(P, 1)[None]
    return b


_RUNNER = None


def _make_runner(nc):
    """Build a cached jitted SPMD executor for the program (the stock
    run_bass_kernel_spmd re-traces and re-jits on every call, which costs
    seconds of host time per launch; this path jits once). Device output
    buffers are recycled across calls so the zeros upload happens once."""
    import jax
    import numpy as _np
    import concourse.mybir as mybir
    from concourse import bass2jax as b2j
    from jax.experimental.shard_map import shard_map
    from jax.sharding import Mesh, PartitionSpec

    b2j.install_neuronx_cc_hook()

    partition_name = nc.partition_id_tensor.name if nc.partition_id_tensor else None
    in_names, out_names, out_avals, zero_outs = [], [], [], []
    for alloc in nc.m.functions[0].allocations:
        if not isinstance(alloc, mybir.MemoryLocationSet):
            continue
        name = alloc.memorylocations[0].name
        if alloc.kind == "ExternalInput":
            if name != partition_name:
                in_names.append(name)
        elif alloc.kind == "ExternalOutput":
            shape = tuple(alloc.tensor_shape)
            dtype = mybir.dt.np(alloc.dtype)
            out_names.append(name)
            out_avals.append(jax.core.ShapedArray(shape, dtype))
            zero_outs.append(_np.zeros(shape, dtype))
    n_params = len(in_names)
    all_names = list(in_names) + list(out_names)
    if partition_name is not None:
        all_names.append(partition_name)
    donate = tuple(range(n_params, n_params + len(out_names)))

    def _body(*args):
        operands = list(args)
        if partition_name is not None:
            operands.append(b2j.partition_id_tensor())
        return tuple(
            b2j._bass_exec_p.bind(
                *operands,
                out_avals=tuple(out_avals),
                in_names=tuple(all_names),
                out_names=tuple(out_names),
                lowering_input_output_aliases=(),
                sim_require_finite=True,
                sim_require_nnan=True,
                nc=nc,
            )
        )

    devices = jax.devices()[:NCORES]
    mesh = Mesh(_np.asarray(devices), ("core",))
    nio = n_params + len(out_names)
    sharded = jax.jit(
        shard_map(
            _body,
            mesh=mesh,
            in_specs=(PartitionSpec("core"),) * nio,
            out_specs=(PartitionSpec("core"),) * len(out_names),
            check_rep=False,
        ),
        donate_argnums=donate,
        keep_unused=True,
    )

    out_cache = []

    def run(in_bufs):
        concat_in = [
            in_bufs[name].reshape(-1, *in_bufs[name].shape[2:]) for name in in_names
        ]
        if out_cache:
            outs_in = list(out_cache)
            out_cache.clear()
        else:
            outs_in = [
                _np.zeros((NCORES * z.shape[0], *z.shape[1:]), z.dtype)
                for z in zero_outs
            ]
        out_arrs = sharded(*concat_in, *outs_in)
        res = {
            name: _np.asarray(out_arrs[i]).reshape(NCORES, *out_avals[i].shape)
            for i, name in enumerate(out_names)
        }
        out_cache.extend(out_arrs)
        return res

    return run


def kernel(x, Lr, Lc, theta, bias):
    global _BUILT, _RUNNER
    if _BUILT is None:
        _BUILT = _build_program()
    if _RUNNER is None:
        _RUNNER = _make_runner(_BUILT)
    in_bufs = _fill_inputs(
        np.asarray(x, dtype=np.float32),
        np.asarray(Lr, dtype=np.float32),
        np.asarray(Lc, dtype=np.float32),
        np.asarray(theta, dtype=np.float32),
        np.asarray(bias, dtype=np.float32),
    )
    res = _RUNNER(in_bufs)
    oc = res["outc"]  # (NCORES, OUT, ML, N) bf16
    out = np.empty((OUT, M, N), np.float32)
    for c in range(NCORES):
        out[:, c * ML : (c + 1) * ML, :] = oc[c]
    return out


# revision 12
# speedup vs baseline: 4.0315x; 4.0315x over previous
"""Trainium2 Bass kernel for nn_BilinearChebConv (bilinear Chebyshev graph conv).

out[o] = sum_{i,j} theta[i,j,0,o] * T_i(Lr) @ x @ T_j(Lc) + bias[o]

Structure (per core c, rows m_c = c*192 .. c*192+191):
    B_i = T_i(Lr)[:, m_c]            (thin column-slice Chebyshev recursion)
    W_0 = x^T @ [B_0 .. B_4]         (n on partitions, (i, m) stacked on free)
    W_j = 2 Lc W_{j-1} - W_{j-2}     (all 5 i-chains in one recursion)
    out[o, m_c, n] = sum_ij theta[ijo] W_j[n, (i, m)] + bias[o]

v3 changes vs v2:
  - theta stage quarters cover contiguous 384-column n-ranges (quarter q
    owns wd blocks 3q..3q+2), so PSUM partition (q,o) rows map to runs of
    contiguous output columns; the device emits out[OUT, 192, 1536]
    directly and host reassembly is a big-block slice assign (~50ms
    instead of a 1.6s 6-D gather).
  - Lr/Lc uploaded unscaled; the Chebyshev 2x lives in the recursion's
    scalar_tensor_tensor scalar. b01 replaced by a static identity-slice
    input (built once at import) + a thin Lr column slice; negi removed.
  - host prep uses persistent pre-concatenated buffers and assignment
    casts (numpy assignment cast is vectorized, .astype is not).
  - output device buffers are recycled across calls (donated), so the
    151MB zeros upload happens only on the first call.
"""

import sys

sys.path.insert(0, "/opt/trn_rl_repo")

import numpy as np
import ml_dtypes

M = 1536
N = 1536
NCORES = 8
ML = M // NCORES          # 192 rows per core
MT = 64                   # max m-stage width
MTS = [64, 64, 64]
OFFS = [0, 64, 128]
NST = len(MTS)
OUT = 32
KB = M // 128             # 12 partition blocks
P = 128
BP = ML                   # per-i column block in B (192, no padding in bf16)

_BF = ml_dtypes.bfloat16
_BUILT = None


def _build_program():
    import concourse.bacc as bacc
    import concourse.mybir as mybir
    from concourse import tile

    F32 = mybir.dt.float32
    BF16 = mybir.dt.bfloat16

    nc = bacc.Bacc(num_devices=NCORES)

    x_d = nc.dram_tensor("x", [M, N], BF16, kind="ExternalInput")
    lr_d = nc.dram_tensor("lr", [M, M], BF16, kind="ExternalInput")
    lc_d = nc.dram_tensor("lc", [N, N], BF16, kind="ExternalInput")
    # ec: per-core identity column slice E_c = I[:, m_c] (static)
    ec_d = nc.dram_tensor("ec", [M, BP], BF16, kind="ExternalInput")
    # lrc1: Lr[:, m_c] (B_1 of the row recursion)
    lrc1_d = nc.dram_tensor("lrc1", [M, BP], BF16, kind="ExternalInput")
    # host-transposed x rows for this core: x^T[:, m_c]  (W0's i=0 segment)
    xct_d = nc.dram_tensor("xct", [N, ML], BF16, kind="ExternalInput")
    thf_d = nc.dram_tensor("thetaf", [100, 4 * OUT], BF16, kind="ExternalInput")
    bias_d = nc.dram_tensor("biasr", [P, 1], F32, kind="ExternalInput")
    # W_{0..4} spill per stage: [nb][j][i][n][m] — (j,i) adjacent and (n,m)
    # contiguous per (j,i), so each theta gather is ONE fat 25-partition DMA
    wd_s = [
        nc.dram_tensor(f"wds{s}", [KB, 5, 5, P, MTS[s]], BF16, kind="Internal")
        for s in range(NST)
    ]
    # direct per-core output layout; host does out[:, m_c, :] = outc[c]
    out_d = nc.dram_tensor("outc", [OUT, ML, N], BF16, kind="ExternalOutput")

    with tile.TileContext(nc) as tc:
        with tc.tile_pool(name="const", bufs=1) as constp:
            thsb = constp.tile([100, 4 * OUT], BF16, tag="thf")
            nc.sync.dma_start(thsb[:], thf_d[:])
            biast = constp.tile([P, 1], F32, tag="bias")
            nc.sync.dma_start(biast[:], bias_d[:])

            # w0res persists from Phase R into Phase C
            with tc.tile_pool(name="w0res", bufs=1) as w0p:
                w0res = []
                for nb in range(KB):
                    w0res.append(
                        w0p.tile([P, 5 * ML], BF16, tag=f"w0_{nb}", name=f"w0res{nb}")
                    )

                # ---------------- Phase R: row stage ----------------
                with (
                    tc.tile_pool(name="lrp", bufs=1) as lrp,
                    tc.tile_pool(name="bpad", bufs=1) as bpadp,
                    tc.tile_pool(name="xs", bufs=1) as xp,
                    tc.tile_pool(name="brps", bufs=2, space="PSUM") as brps,
                    tc.tile_pool(name="w0ps", bufs=2, space="PSUM") as w0ps,
                ):
                    bt = []
                    for k in range(KB):
                        t_ = bpadp.tile([P, 5 * BP], BF16, tag=f"bp{k}", name=f"bt{k}")
                        [nc.scalar, nc.sync][k % 2].dma_start(
                            t_[:, 0:BP], ec_d[k * P : (k + 1) * P, :]
                        )
                        [nc.sync, nc.scalar][k % 2].dma_start(
                            t_[:, BP : 2 * BP], lrc1_d[k * P : (k + 1) * P, :]
                        )
                        bt.append(t_)
                    xt = []
                    for kp in range(KB // 2):
                        t_ = xp.tile([P, 2 * N], BF16, tag=f"x{kp}", name=f"xt{kp}")
                        dst = t_.rearrange("p (k n) -> p k n", k=2)
                        src = x_d[2 * kp * P : (2 * kp + 2) * P, :].rearrange(
                            "(k p) n -> p k n", k=2
                        )
                        nc.gpsimd.dma_start(dst, src)
                        xt.append(t_[:, 0:N])
                        xt.append(t_[:, N : 2 * N])
                    lrt = []
                    for kp in range(KB // 2):
                        t_ = lrp.tile([P, 2 * M], BF16, tag=f"lr{kp}", name=f"lrt{kp}")
                        dst = t_.rearrange("p (k n) -> p k n", k=2)
                        src = lr_d[2 * kp * P : (2 * kp + 2) * P, :].rearrange(
                            "(k p) n -> p k n", k=2
                        )
                        [nc.sync, nc.scalar][kp % 2].dma_start(dst, src)
                        lrt.append(t_[:, 0:M])
                        lrt.append(t_[:, M : 2 * M])

                    import concourse.mybir as mybir
                    SUB0 = mybir.AluOpType.subtract
                    MULT0 = mybir.AluOpType.mult

                    # W0 segment A: i = 0 comes straight from the host-side
                    # transpose of x's m_c rows; i = 1 (cols 192:384) is the
                    # only matmul here — runs while lr loads / B-rec warm up
                    for nb in range(KB):
                        [nc.sync, nc.scalar][nb % 2].dma_start(
                            w0res[nb][:, 0:BP], xct_d[nb * P : (nb + 1) * P, :]
                        )
                        psA = w0ps.tile([P, BP], F32, tag="w0sA", name="w0psA")
                        for k in range(KB):
                            nc.tensor.matmul(
                                psA[:],
                                lhsT=xt[k][:, nb * P : (nb + 1) * P],
                                rhs=bt[k][:, BP : 2 * BP],
                                start=(k == 0),
                                stop=(k == KB - 1),
                            )
                        nc.vector.tensor_copy(w0res[nb][:, BP : 2 * BP], psA[:])

                    # B recursion: B_i = 2 Lr @ B_{i-1} - B_{i-2}, i = 2..4
                    for i in range(2, 5):
                        for p in range(KB):
                            ps = brps.tile([P, BP], F32, tag="brec", name="brps")
                            for k in range(KB):
                                nc.tensor.matmul(
                                    ps[:],
                                    lhsT=lrt[k][:, p * P : (p + 1) * P],
                                    rhs=bt[k][:, (i - 1) * BP : i * BP],
                                    start=(k == 0),
                                    stop=(k == KB - 1),
                                )
                            nc.vector.scalar_tensor_tensor(
                                bt[p][:, i * BP : (i + 1) * BP],
                                ps[:],
                                2.0,
                                bt[p][:, (i - 2) * BP : (i - 1) * BP],
                                MULT0,
                                SUB0,
                            )

                    # W0 segment B: i = 2..4 (cols 384:960), after B-rec
                    segs = [(2 * BP, 512), (2 * BP + 512, 3 * BP - 512)]
                    for nb in range(KB):
                        pss = [
                            w0ps.tile([P, sz], F32, tag=f"w0s{si}", name=f"w0ps{si}")
                            for si, (_, sz) in enumerate(segs)
                        ]
                        for k in range(KB):
                            for ps, (off, sz) in zip(pss, segs):
                                nc.tensor.matmul(
                                    ps[:],
                                    lhsT=xt[k][:, nb * P : (nb + 1) * P],
                                    rhs=bt[k][:, off : off + sz],
                                    start=(k == 0),
                                    stop=(k == KB - 1),
                                )
                        for ps, (off, sz) in zip(pss, segs):
                            nc.vector.tensor_copy(w0res[nb][:, off : off + sz], ps[:])
                        w0v3 = w0res[nb].rearrange("n (i m) -> n i m", i=5)
                        for si in range(NST):
                            [nc.sync, nc.scalar, nc.gpsimd][(nb + si) % 3].dma_start(
                                wd_s[si][nb, 0].rearrange("i n m -> n i m"),
                                w0v3[:, :, OFFS[si] : OFFS[si] + MTS[si]],
                            )

                # ---------------- Phase C: column stage + theta ----------------
                with (
                    tc.tile_pool(name="lcp", bufs=1) as lcp,
                    tc.tile_pool(name="wp", bufs=4) as wp,
                    tc.tile_pool(name="zfp", bufs=8) as zfp,
                    tc.tile_pool(name="evp", bufs=1) as evp,
                    tc.tile_pool(name="wps", bufs=2, space="PSUM") as wps,
                    tc.tile_pool(name="thps", bufs=1, space="PSUM") as thps,
                ):
                    lct = []
                    for k in range(KB):
                        t_ = lcp.tile([P, N], BF16, tag=f"lc{k}", name=f"lct{k}")
                        [nc.sync, nc.scalar, nc.gpsimd][k % 3].dma_start(
                            t_[:], lc_d[k * P : (k + 1) * P, :]
                        )
                        lct.append(t_)

                    import concourse.mybir as mybir
                    SUB = mybir.AluOpType.subtract
                    MULT = mybir.AluOpType.mult

                    # j=4 chunk order: complete quarter-groups {u,3+u,6+u,9+u}
                    # early so theta steps can interleave
                    J4ORDER = [0, 3, 6, 9, 1, 4, 7, 10, 2, 5, 8, 11]

                    def jrec_chunks(s):
                        """Yield (j, nb, emit_fn) for stage s's j-recursion."""
                        mt, off = MTS[s], OFFS[s]
                        wcur = [[None] * KB for _ in range(5)]
                        w0v = [
                            w0res[k]
                            .rearrange("n (i m) -> n i m", i=5)[:, :, off : off + mt]
                            for k in range(KB)
                        ]

                        for j in range(1, 5):
                            nbs = J4ORDER if j == 4 else range(KB)
                            for nb in nbs:
                                def chunk(j=j, nb=nb):
                                    psf = wps.tile(
                                        [P, 5 * MT], F32, tag="wrec", name=f"wps{j}"
                                    )
                                    ps = psf[:, 0 : 5 * mt]
                                    rhs_prev = (
                                        w0v
                                        if j == 1
                                        else [w[:, 0 : 5 * mt] for w in wcur[j - 1]]
                                    )
                                    for k in range(KB):
                                        nc.tensor.matmul(
                                            ps,
                                            lhsT=lct[k][:, nb * P : (nb + 1) * P],
                                            rhs=rhs_prev[k],
                                            start=(k == 0),
                                            stop=(k == KB - 1),
                                        )
                                    w = wp.tile(
                                        [P, 5 * MT], BF16, tag=f"w_{nb}",
                                        name=f"w{j}t{nb}",
                                    )
                                    if j == 1:
                                        nc.vector.tensor_copy(w[:, 0 : 5 * mt], ps)
                                    else:
                                        prev2 = (
                                            w0v[nb]
                                            if j == 2
                                            else wcur[j - 2][nb][
                                                :, 0 : 5 * mt
                                            ].rearrange("n (i m) -> n i m", i=5)
                                        )
                                        nc.vector.scalar_tensor_tensor(
                                            w[:, 0 : 5 * mt].rearrange(
                                                "n (i m) -> n i m", i=5
                                            ),
                                            ps.rearrange("n (i m) -> n i m", i=5),
                                            2.0,
                                            prev2,
                                            MULT,
                                            SUB,
                                        )
                                    wcur[j][nb] = w
                                    eng = nc.sync if nb % 2 == 0 else nc.scalar
                                    eng.dma_start(
                                        wd_s[s][nb, j].rearrange("i n m -> n i m"),
                                        w[:, 0 : 5 * mt],
                                    )
                                yield j, nb, chunk

                    engs = [nc.gpsimd, nc.sync, nc.scalar]

                    def gather(s, t, zft):
                        # theta step t covers n = q*384 + t*32 .. +32 per
                        # quarter q, i.e. wd block 3q + t//4, cols (t%4)*32
                        mt = MTS[s]
                        u, v = t // 4, t % 4
                        zff = zfp.tile([100, 32 * MT], BF16, tag="zf", name=f"zf{t%8}")
                        zf = zff[:, 0 : 32 * mt]
                        for q in range(4):
                            src = wd_s[s][3 * q + u].rearrange(
                                "j i n m -> (j i) n m"
                            )
                            zv = zf.rearrange("p (n m) -> p n m", n=32)[
                                q * 25 : (q + 1) * 25
                            ]
                            engs[(4 * t + q) % 3].dma_start(
                                zv, src[:, v * 32 : (v + 1) * 32, :]
                            )
                        zft[t] = zff

                    def theta_compute(s, t, ev, zft):
                        mt = MTS[s]
                        # ev free layout (m mt, t KB, nq 32)
                        evv = ev.rearrange("p (ml t n) -> p ml t n", t=KB, n=32)
                        zff = zft.pop(t)
                        zm = zff[:, 0 : 32 * mt].rearrange("p (n m) -> p m n", n=32)
                        for kk2 in range(mt // 32):
                            ps = thps.tile(
                                [P, 1024], F32, tag=f"th{kk2%2}", name=f"thps{kk2}"
                            )
                            for h in range(2):
                                kk = kk2 * 2 + h
                                nc.tensor.matmul(
                                    ps[:, h * 512 : (h + 1) * 512],
                                    lhsT=thsb[:],
                                    rhs=zm[:, kk * 16 : (kk + 1) * 16, :],
                                    start=True,
                                    stop=True,
                                )
                            dst = evv[:, kk2 * 32 : (kk2 + 1) * 32, t, :]
                            srcp = ps.rearrange("p (m n) -> p m n", m=32)
                            nc.vector.tensor_scalar_add(dst, srcp, biast[:])

                    def out_dmas(s, ev, piece, npieces):
                        # partition (q,o) holds n = q*384 + t*32 + nq —
                        # contiguous runs in the [OUT, ML, N] output; one
                        # DMA per quarter (32-partition SBUF slice)
                        mt = MTS[s]
                        tpp = KB // npieces
                        t0, t1 = piece * tpp, (piece + 1) * tpp
                        evq = ev.rearrange("p (ml t n) -> p ml t n", t=KB, n=32)
                        for q in range(4):
                            dst = out_d[
                                :,
                                OFFS[s] : OFFS[s] + mt,
                                q * 384 : (q + 1) * 384,
                            ].rearrange("o m (t n) -> o m t n", n=32)[:, :, t0:t1, :]
                            src = evq[q * 32 : (q + 1) * 32, 0:mt, t0:t1, :]
                            [nc.scalar, nc.sync, nc.gpsimd][
                                (4 * piece + q) % 3
                            ].dma_start(dst, src)

                    # pipeline: quarter-group u's gathers fire once wd blocks
                    # {u, 3+u, 6+u, 9+u} are spilled; theta steps trail them
                    for s in range(NST):
                        last = s == NST - 1
                        npieces = 3 if last else 1
                        ev = evp.tile([P, MT * KB * 32], BF16, tag="ev", name=f"ev{s}")
                        zft = {}
                        done = [0]

                        def step(s=s, ev=ev, zft=zft, done=done, npieces=npieces):
                            t = done[0]
                            done[0] += 1
                            theta_compute(s, t, ev, zft)
                            d = done[0]
                            if d % (KB // npieces) == 0 and d < KB:
                                out_dmas(s, ev, d // (KB // npieces) - 1, npieces)

                        k = 0
                        for j, nb, chunk in jrec_chunks(s):
                            chunk()
                            if j == 4:
                                k += 1
                                if k % 4 == 0:
                                    u = k // 4 - 1
                                    for t in range(4 * u, 4 * u + 4):
                                        gather(s, t, zft)
                                if k > 4 and done[0] < 4 * (k // 4):
                                    step()
                        while done[0] < KB:
                            step()
                        out_dmas(s, ev, npieces - 1, npieces)

    nc.finalize()
    return nc


_HOST = None


def _host_state():
    """Persistent pre-concatenated input buffers (built once)."""
    global _HOST
    if _HOST is not None:
        return _HOST
    bufs = {
        "x": np.empty((M, N), _BF),
        "lr": np.empty((M, M), _BF),
        "lc": np.empty((N, N), _BF),
        "ec": np.zeros((NCORES, M, BP), _BF),
        "lrc1": np.empty((NCORES, M, BP), _BF),
        "xct": np.empty((NCORES, N, ML), _BF),
        "thetaf": np.zeros((NCORES, 100, 4 * OUT), _BF),
        "biasr": np.empty((NCORES, P, 1), np.float32),
    }
    one = np.ones((ML,), np.float32)
    for c in range(NCORES):
        bufs["ec"][c, c * ML : (c + 1) * ML, :][np.arange(ML), np.arange(ML)] = one
    _HOST = bufs
    return bufs


def _fill_inputs(x, Lr, Lc, theta, bias):
    b = _host_state()
    x2 = x.reshape(M, N)
    b["x"][...] = x2
    b["lr"][...] = Lr
    b["lc"][...] = Lc
    xT = x2.T
    for c in range(NCORES):
        cols = slice(c * ML, (c + 1) * ML)
        b["lrc1"][c] = Lr[:, cols]
        b["xct"][c] = xT[:, cols]
    thf25 = np.zeros((25, OUT), np.float32)
    th = theta.reshape(5, 5, OUT)
    for i in range(5):
        for j in range(5):
            thf25[j * 5 + i] = th[i, j]
    for q in range(4):
        b["thetaf"][:, q * 25 : (q + 1) * 25, q * OUT : (q + 1) * OUT] = thf25[None]
    # thetaf/biasr are tiny; keep them per-core sharded (concat of copies)
    b["biasr"][...] = np.tile(bias.astype(np.float32), 4).reshape(P, 1)[None]
    return b


_RUNNER = None


def _make_runner(nc):
    """Build a cached jitted SPMD executor for the program (the stock
    run_bass_kernel_spmd re-traces and re-jits on every call, which costs
    seconds of host time per launch; this path jits once). Device output
    buffers are recycled across calls so the zeros upload happens once."""
    import jax
    import numpy as _np
    import concourse.mybir as mybir
    from concourse import bass2jax as b2j
    from jax.experimental.shard_map import shard_map
    from jax.sharding import Mesh, PartitionSpec

    b2j.install_neuronx_cc_hook()

    partition_name = nc.partition_id_tensor.name if nc.partition_id_tensor else None
    in_names, out_names, out_avals, zero_outs = [], [], [], []
    for alloc in nc.m.functions[0].allocations:
        if not isinstance(alloc, mybir.MemoryLocationSet):
            continue
        name = alloc.memorylocations[0].name
        if alloc.kind == "ExternalInput":
            if name != partition_name:
                in_names.append(name)
        elif alloc.kind == "ExternalOutput":
            shape = tuple(alloc.tensor_shape)
            dtype = mybir.dt.np(alloc.dtype)
            out_names.append(name)
            out_avals.append(jax.core.ShapedArray(shape, dtype))
            zero_outs.append(_np.zeros(shape, dtype))
    n_params = len(in_names)
    all_names = list(in_names) + list(out_names)
    if partition_name is not None:
        all_names.append(partition_name)
    donate = tuple(range(n_params, n_params + len(out_names)))

    def _body(*args):
        operands = list(args)
        if partition_name is not None:
            operands.append(b2j.partition_id_tensor())
        return tuple(
            b2j._bass_exec_p.bind(
                *operands,
                out_avals=tuple(out_avals),
                in_names=tuple(all_names),
                out_names=tuple(out_names),
                lowering_input_output_aliases=(),
                sim_require_finite=True,
                sim_require_nnan=True,
                nc=nc,
            )
        )

    devices = jax.devices()[:NCORES]
    mesh = Mesh(_np.asarray(devices), ("core",))
    # x/lr/lc are identical on every core: replicate instead of concatenating
    # 8 host-side copies
    rep_names = {"x", "lr", "lc"}
    in_specs = tuple(
        PartitionSpec() if name in rep_names else PartitionSpec("core")
        for name in in_names
    ) + (PartitionSpec("core"),) * len(out_names)
    sharded = jax.jit(
        shard_map(
            _body,
            mesh=mesh,
            in_specs=in_specs,
            out_specs=(PartitionSpec("core"),) * len(out_names),
            check_rep=False,
        ),
        donate_argnums=donate,
        keep_unused=True,
    )

    out_cache = []

    def run(in_bufs):
        concat_in = [
            in_bufs[name]
            if name in rep_names
            else in_bufs[name].reshape(-1, *in_bufs[name].shape[2:])
            for name in in_names
        ]
        if out_cache:
            outs_in = list(out_cache)
            out_cache.clear()
        else:
            outs_in = [
                _np.zeros((NCORES * z.shape[0], *z.shape[1:]), z.dtype)
                for z in zero_outs
            ]
        out_arrs = sharded(*concat_in, *outs_in)
        res = {
            name: _np.asarray(out_arrs[i]).reshape(NCORES, *out_avals[i].shape)
            for i, name in enumerate(out_names)
        }
        out_cache.extend(out_arrs)
        return res

    return run


def kernel(x, Lr, Lc, theta, bias):
    global _BUILT, _RUNNER
    if _BUILT is None:
        _BUILT = _build_program()
    if _RUNNER is None:
        _RUNNER = _make_runner(_BUILT)
    in_bufs = _fill_inputs(
        np.asarray(x, dtype=np.float32),
        np.asarray(Lr, dtype=np.float32),
        np.asarray(Lc, dtype=np.float32),
        np.asarray(theta, dtype=np.float32),
        np.asarray(bias, dtype=np.float32),
    )
    res = _RUNNER(in_bufs)
    oc = res["outc"]  # (NCORES, OUT, ML, N) bf16
    out = np.empty((OUT, M, N), np.float32)
    for c in range(NCORES):
        out[:, c * ML : (c + 1) * ML, :] = oc[c]
    return out


# revision 31
# speedup vs baseline: 4.5889x; 1.1383x over previous
"""Trainium2 Bass kernel for nn_BilinearChebConv (bilinear Chebyshev graph conv).

out[o] = sum_{i,j} theta[i,j,0,o] * T_i(Lr) @ x @ T_j(Lc) + bias[o]

Structure (per core c, rows m_c = c*192 .. c*192+191):
    B_i = T_i(Lr)[:, m_c]            (thin column-slice Chebyshev recursion)
    W_0 = x^T @ [B_0 .. B_4]         (n on partitions, (i, m) stacked on free)
    W_j = 2 Lc W_{j-1} - W_{j-2}     (all 5 i-chains in one recursion)
    out[o, m_c, n] = sum_ij theta[ijo] W_j[n, (i, m)] + bias[o]

v3 changes vs v2:
  - theta stage quarters cover contiguous 384-column n-ranges (quarter q
    owns wd blocks 3q..3q+2), so PSUM partition (q,o) rows map to runs of
    contiguous output columns; the device emits out[OUT, 192, 1536]
    directly and host reassembly is a big-block slice assign (~50ms
    instead of a 1.6s 6-D gather).
  - Lr/Lc uploaded unscaled; the Chebyshev 2x lives in the recursion's
    scalar_tensor_tensor scalar. b01 replaced by a static identity-slice
    input (built once at import) + a thin Lr column slice; negi removed.
  - host prep uses persistent pre-concatenated buffers and assignment
    casts (numpy assignment cast is vectorized, .astype is not).
  - output device buffers are recycled across calls (donated), so the
    151MB zeros upload happens only on the first call.
"""

import sys

sys.path.insert(0, "/opt/trn_rl_repo")

import numpy as np
import ml_dtypes

M = 1536
N = 1536
NCORES = 8
ML = M // NCORES          # 192 rows per core
MT = 64                   # max m-stage width
MTS = [64, 64, 64]
OFFS = [0, 64, 128]
NST = len(MTS)
OUT = 32
KB = M // 128             # 12 partition blocks
P = 128
BP = ML                   # per-i column block in B (192, no padding in bf16)

_BF = ml_dtypes.bfloat16
_BUILT = None


def _build_program():
    import concourse.bacc as bacc
    import concourse.mybir as mybir
    from concourse import tile

    F32 = mybir.dt.float32
    BF16 = mybir.dt.bfloat16

    nc = bacc.Bacc(num_devices=NCORES)

    # x/lr/lc arrive row-sharded (core c holds rows m_c); a device-side
    # AllGather rebuilds the full matrices, cutting host upload 8x
    x_d = nc.dram_tensor("x", [ML, N], BF16, kind="ExternalInput")
    lr_d = nc.dram_tensor("lr", [ML, M], BF16, kind="ExternalInput")
    lc_d = nc.dram_tensor("lc", [ML, N], BF16, kind="ExternalInput")
    # ec: per-core identity column slice E_c = I[:, m_c] (static, kept
    # device-resident across calls)
    ec_d = nc.dram_tensor("ec", [M, BP], BF16, kind="ExternalInput")
    thf_d = nc.dram_tensor("thetaf", [100, 4 * OUT], BF16, kind="ExternalInput")
    bias_d = nc.dram_tensor("biasr", [P, 1], F32, kind="ExternalInput")
    # collective staging (collectives cannot read IO tensors); x|lr|lc are
    # fused into ONE AllGather — the collective cost model's bandwidth
    # ramps with size, so one 14MB gather beats three 4.7MB ones
    W3 = 2 * N + M
    st_d = nc.dram_tensor("st", [ML, W3], BF16, kind="Internal")
    ff_d = nc.dram_tensor("ff", [M, W3], BF16, kind="Internal", addr_space="Shared")
    # W_{0..4} spill per stage: [nb][j][i][n][m] — (j,i) adjacent and (n,m)
    # contiguous per (j,i), so each theta gather is ONE fat 25-partition DMA
    wd_s = [
        nc.dram_tensor(f"wds{s}", [KB, 5, 5, P, MTS[s]], BF16, kind="Internal")
        for s in range(NST)
    ]
    # direct per-core output layout; host does out[:, m_c, :] = outc[c]
    out_d = nc.dram_tensor("outc", [OUT, ML, N], BF16, kind="ExternalOutput")

    RG = [list(range(NCORES))]

    with tile.TileContext(nc) as tc:
        # stage + AllGather the three sharded matrices; bt/theta/bias loads
        # below overlap the collectives
        nc.sync.dma_start(st_d[:, 0:N], x_d[:, :])
        nc.scalar.dma_start(st_d[:, N : N + M], lr_d[:, :])
        nc.sync.dma_start(st_d[:, N + M : W3], lc_d[:, :])
        import concourse.mybir as _mybir
        nc.gpsimd.collective_compute(
            "AllGather", _mybir.AluOpType.bypass, RG, [st_d[:, :]], [ff_d[:, :]]
        )

        with tc.tile_pool(name="const", bufs=1) as constp:
            thsb = constp.tile([100, 4 * OUT], BF16, tag="thf")
            nc.sync.dma_start(thsb[:], thf_d[:])
            biast = constp.tile([P, 1], F32, tag="bias")
            nc.sync.dma_start(biast[:], bias_d[:])

            # w0res persists from Phase R into Phase C
            with tc.tile_pool(name="w0res", bufs=1) as w0p:
                w0res = []
                for nb in range(KB):
                    w0res.append(
                        w0p.tile([P, 5 * ML], BF16, tag=f"w0_{nb}", name=f"w0res{nb}")
                    )

                # ---------------- Phase R: row stage ----------------
                with (
                    tc.tile_pool(name="lrp", bufs=1) as lrp,
                    tc.tile_pool(name="bpad", bufs=1) as bpadp,
                    tc.tile_pool(name="xs", bufs=1) as xp,
                    tc.tile_pool(name="brps", bufs=2, space="PSUM") as brps,
                    tc.tile_pool(name="w0ps", bufs=2, space="PSUM") as w0ps,
                ):
                    bt = []
                    for k in range(KB):
                        t_ = bpadp.tile([P, 5 * BP], BF16, tag=f"bp{k}", name=f"bt{k}")
                        [nc.scalar, nc.sync][k % 2].dma_start(
                            t_[:, 0:BP], ec_d[k * P : (k + 1) * P, :]
                        )
                        bt.append(t_)
                    xt = []
                    for kp in range(KB // 2):
                        t_ = xp.tile([P, 2 * N], BF16, tag=f"x{kp}", name=f"xt{kp}")
                        dst = t_.rearrange("p (k n) -> p k n", k=2)
                        src = ff_d[2 * kp * P : (2 * kp + 2) * P, 0:N].rearrange(
                            "(k p) n -> p k n", k=2
                        )
                        [nc.scalar, nc.sync][kp % 2].dma_start(dst, src)
                        xt.append(t_[:, 0:N])
                        xt.append(t_[:, N : 2 * N])
                    lrt = []
                    for kp in range(KB // 2):
                        t_ = lrp.tile([P, 2 * M], BF16, tag=f"lr{kp}", name=f"lrt{kp}")
                        dst = t_.rearrange("p (k n) -> p k n", k=2)
                        src = ff_d[
                            2 * kp * P : (2 * kp + 2) * P, N : N + M
                        ].rearrange("(k p) n -> p k n", k=2)
                        [nc.sync, nc.scalar][kp % 2].dma_start(dst, src)
                        lrt.append(t_[:, 0:M])
                        lrt.append(t_[:, M : 2 * M])

                    import concourse.mybir as mybir
                    SUB0 = mybir.AluOpType.subtract
                    MULT0 = mybir.AluOpType.mult

                    # B_1 = Lr^T @ E_c (column slice of Lr^T; the whole row
                    # recursion runs on Lr^T, which is exactly what the
                    # einsum's row-transform needs — no symmetry assumption)
                    for p in range(KB):
                        ps = brps.tile([P, BP], F32, tag="brec", name="b1ps")
                        for k in range(KB):
                            nc.tensor.matmul(
                                ps[:],
                                lhsT=lrt[k][:, p * P : (p + 1) * P],
                                rhs=bt[k][:, 0:BP],
                                start=(k == 0),
                                stop=(k == KB - 1),
                            )
                        nc.vector.tensor_copy(bt[p][:, BP : 2 * BP], ps[:])

                    # W0 segment A: i = 0 and i = 1 together, one 384-col
                    # matmul per k (rhs = [E_c | B_1])
                    for nb in range(KB):
                        psA = w0ps.tile([P, 2 * BP], F32, tag="w0sA", name="w0psA")
                        for k in range(KB):
                            nc.tensor.matmul(
                                psA[:],
                                lhsT=xt[k][:, nb * P : (nb + 1) * P],
                                rhs=bt[k][:, 0 : 2 * BP],
                                start=(k == 0),
                                stop=(k == KB - 1),
                            )
                        nc.vector.tensor_copy(w0res[nb][:, 0 : 2 * BP], psA[:])

                    # B recursion: B_i = 2 Lr @ B_{i-1} - B_{i-2}, i = 2..4
                    for i in range(2, 5):
                        for p in range(KB):
                            ps = brps.tile([P, BP], F32, tag="brec", name="brps")
                            for k in range(KB):
                                nc.tensor.matmul(
                                    ps[:],
                                    lhsT=lrt[k][:, p * P : (p + 1) * P],
                                    rhs=bt[k][:, (i - 1) * BP : i * BP],
                                    start=(k == 0),
                                    stop=(k == KB - 1),
                                )
                            nc.vector.scalar_tensor_tensor(
                                bt[p][:, i * BP : (i + 1) * BP],
                                ps[:],
                                2.0,
                                bt[p][:, (i - 2) * BP : (i - 1) * BP],
                                MULT0,
                                SUB0,
                            )

                    # W0 segment B: i = 2..4 (cols 384:960), after B-rec
                    segs = [(2 * BP, 512), (2 * BP + 512, 3 * BP - 512)]
                    for nb in range(KB):
                        pss = [
                            w0ps.tile([P, sz], F32, tag=f"w0s{si}", name=f"w0ps{si}")
                            for si, (_, sz) in enumerate(segs)
                        ]
                        for k in range(KB):
                            for ps, (off, sz) in zip(pss, segs):
                                nc.tensor.matmul(
                                    ps[:],
                                    lhsT=xt[k][:, nb * P : (nb + 1) * P],
                                    rhs=bt[k][:, off : off + sz],
                                    start=(k == 0),
                                    stop=(k == KB - 1),
                                )
                        for ps, (off, sz) in zip(pss, segs):
                            nc.vector.tensor_copy(w0res[nb][:, off : off + sz], ps[:])
                        w0v3 = w0res[nb].rearrange("n (i m) -> n i m", i=5)
                        for si in range(NST):
                            [nc.sync, nc.scalar, nc.gpsimd][(nb + si) % 3].dma_start(
                                wd_s[si][nb, 0].rearrange("i n m -> n i m"),
                                w0v3[:, :, OFFS[si] : OFFS[si] + MTS[si]],
                            )

                # ---------------- Phase C: column stage + theta ----------------
                with (
                    tc.tile_pool(name="lcp", bufs=1) as lcp,
                    tc.tile_pool(name="wp", bufs=4) as wp,
                    tc.tile_pool(name="zfp", bufs=8) as zfp,
                    tc.tile_pool(name="evp", bufs=1) as evp,
                    tc.tile_pool(name="wps", bufs=2, space="PSUM") as wps,
                    tc.tile_pool(name="thps", bufs=1, space="PSUM") as thps,
                ):
                    lct = []
                    for k in range(KB):
                        t_ = lcp.tile([P, N], BF16, tag=f"lc{k}", name=f"lct{k}")
                        [nc.sync, nc.scalar][k % 2].dma_start(
                            t_[:], ff_d[k * P : (k + 1) * P, N + M : W3]
                        )
                        lct.append(t_)

                    import concourse.mybir as mybir
                    SUB = mybir.AluOpType.subtract
                    MULT = mybir.AluOpType.mult

                    # j=4 chunk order: complete quarter-groups {u,3+u,6+u,9+u}
                    # early so theta steps can interleave
                    J4ORDER = [0, 3, 6, 9, 1, 4, 7, 10, 2, 5, 8, 11]

                    def jrec_chunks(s):
                        """Yield (j, nb, emit_fn) for stage s's j-recursion."""
                        mt, off = MTS[s], OFFS[s]
                        wcur = [[None] * KB for _ in range(5)]
                        w0v = [
                            w0res[k]
                            .rearrange("n (i m) -> n i m", i=5)[:, :, off : off + mt]
                            for k in range(KB)
                        ]

                        for j in range(1, 5):
                            nbs = J4ORDER if j == 4 else range(KB)
                            for nb in nbs:
                                def chunk(j=j, nb=nb):
                                    psf = wps.tile(
                                        [P, 5 * MT], F32, tag="wrec", name=f"wps{j}"
                                    )
                                    ps = psf[:, 0 : 5 * mt]
                                    rhs_prev = (
                                        w0v
                                        if j == 1
                                        else [w[:, 0 : 5 * mt] for w in wcur[j - 1]]
                                    )
                                    for k in range(KB):
                                        nc.tensor.matmul(
                                            ps,
                                            lhsT=lct[k][:, nb * P : (nb + 1) * P],
                                            rhs=rhs_prev[k],
                                            start=(k == 0),
                                            stop=(k == KB - 1),
                                        )
                                    w = wp.tile(
                                        [P, 5 * MT], BF16, tag=f"w_{nb}",
                                        name=f"w{j}t{nb}",
                                    )
                                    if j == 1:
                                        nc.vector.tensor_copy(w[:, 0 : 5 * mt], ps)
                                    else:
                                        prev2 = (
                                            w0v[nb]
                                            if j == 2
                                            else wcur[j - 2][nb][
                                                :, 0 : 5 * mt
                                            ].rearrange("n (i m) -> n i m", i=5)
                                        )
                                        nc.vector.scalar_tensor_tensor(
                                            w[:, 0 : 5 * mt].rearrange(
                                                "n (i m) -> n i m", i=5
                                            ),
                                            ps.rearrange("n (i m) -> n i m", i=5),
                                            2.0,
                                            prev2,
                                            MULT,
                                            SUB,
                                        )
                                    wcur[j][nb] = w
                                    eng = nc.sync if nb % 2 == 0 else nc.scalar
                                    eng.dma_start(
                                        wd_s[s][nb, j].rearrange("i n m -> n i m"),
                                        w[:, 0 : 5 * mt],
                                    )
                                yield j, nb, chunk

                    engs = [nc.gpsimd, nc.sync, nc.scalar]

                    def gather(s, t, zft):
                        # theta step t covers n = q*384 + t*32 .. +32 per
                        # quarter q, i.e. wd block 3q + t//4, cols (t%4)*32
                        mt = MTS[s]
                        u, v = t // 4, t % 4
                        zff = zfp.tile([100, 32 * MT], BF16, tag="zf", name=f"zf{t%8}")
                        zf = zff[:, 0 : 32 * mt]
                        for q in range(4):
                            src = wd_s[s][3 * q + u].rearrange(
                                "j i n m -> (j i) n m"
                            )
                            zv = zf.rearrange("p (n m) -> p n m", n=32)[
                                q * 25 : (q + 1) * 25
                            ]
                            engs[(4 * t + q) % 3].dma_start(
                                zv, src[:, v * 32 : (v + 1) * 32, :]
                            )
                        zft[t] = zff

                    def theta_compute(s, t, ev, zft):
                        mt = MTS[s]
                        # ev free layout (m mt, t KB, nq 32)
                        evv = ev.rearrange("p (ml t n) -> p ml t n", t=KB, n=32)
                        zff = zft.pop(t)
                        zm = zff[:, 0 : 32 * mt].rearrange("p (n m) -> p m n", n=32)
                        for kk2 in range(mt // 32):
                            ps = thps.tile(
                                [P, 1024], F32, tag=f"th{kk2%2}", name=f"thps{kk2}"
                            )
                            for h in range(2):
                                kk = kk2 * 2 + h
                                nc.tensor.matmul(
                                    ps[:, h * 512 : (h + 1) * 512],
                                    lhsT=thsb[:],
                                    rhs=zm[:, kk * 16 : (kk + 1) * 16, :],
                                    start=True,
                                    stop=True,
                                )
                            dst = evv[:, kk2 * 32 : (kk2 + 1) * 32, t, :]
                            srcp = ps.rearrange("p (m n) -> p m n", m=32)
                            nc.vector.tensor_scalar_add(dst, srcp, biast[:])

                    def out_dmas(s, ev, piece, npieces):
                        # partition (q,o) holds n = q*384 + t*32 + nq —
                        # contiguous runs in the [OUT, ML, N] output; one
                        # DMA per quarter (32-partition SBUF slice)
                        mt = MTS[s]
                        tpp = KB // npieces
                        t0, t1 = piece * tpp, (piece + 1) * tpp
                        evq = ev.rearrange("p (ml t n) -> p ml t n", t=KB, n=32)
                        for q in range(4):
                            dst = out_d[
                                :,
                                OFFS[s] : OFFS[s] + mt,
                                q * 384 : (q + 1) * 384,
                            ].rearrange("o m (t n) -> o m t n", n=32)[:, :, t0:t1, :]
                            src = evq[q * 32 : (q + 1) * 32, 0:mt, t0:t1, :]
                            [nc.scalar, nc.sync, nc.gpsimd][
                                (4 * piece + q) % 3
                            ].dma_start(dst, src)

                    # pipeline: quarter-group u's gathers fire once wd blocks
                    # {u, 3+u, 6+u, 9+u} are spilled; theta steps trail them
                    for s in range(NST):
                        last = s == NST - 1
                        npieces = 3 if last else 1
                        ev = evp.tile([P, MT * KB * 32], BF16, tag="ev", name=f"ev{s}")
                        zft = {}
                        done = [0]

                        def step(s=s, ev=ev, zft=zft, done=done, npieces=npieces):
                            t = done[0]
                            done[0] += 1
                            theta_compute(s, t, ev, zft)
                            d = done[0]
                            if d % (KB // npieces) == 0 and d < KB:
                                out_dmas(s, ev, d // (KB // npieces) - 1, npieces)

                        k = 0
                        for j, nb, chunk in jrec_chunks(s):
                            chunk()
                            if j == 4:
                                k += 1
                                if k % 4 == 0:
                                    u = k // 4 - 1
                                    for t in range(4 * u, 4 * u + 4):
                                        gather(s, t, zft)
                                if k > 4 and done[0] < 4 * (k // 4):
                                    step()
                        while done[0] < KB:
                            step()
                        out_dmas(s, ev, npieces - 1, npieces)

    nc.finalize()
    return nc


_HOST = None


def _host_state():
    """Persistent pre-concatenated input buffers (built once)."""
    global _HOST
    if _HOST is not None:
        return _HOST
    bufs = {
        # x/lr/lc are row-sharded on device: the (M, N) buffer split into 8
        # row blocks IS the sharding, so no replication or reshuffle needed
        "x": np.empty((M, N), _BF),
        "lr": np.empty((M, M), _BF),
        "lc": np.empty((N, N), _BF),
        "ec": np.zeros((NCORES, M, BP), _BF),
        "thetaf": np.zeros((NCORES, 100, 4 * OUT), _BF),
        "biasr": np.empty((NCORES, P, 1), np.float32),
    }
    one = np.ones((ML,), np.float32)
    for c in range(NCORES):
        bufs["ec"][c, c * ML : (c + 1) * ML, :][np.arange(ML), np.arange(ML)] = one
    _HOST = bufs
    return bufs


def _fill_inputs(x, Lr, Lc, theta, bias):
    b = _host_state()
    x2 = x.reshape(M, N)
    b["x"][...] = x2
    b["lr"][...] = Lr
    b["lc"][...] = Lc
    thf25 = np.zeros((25, OUT), np.float32)
    th = theta.reshape(5, 5, OUT)
    for i in range(5):
        for j in range(5):
            thf25[j * 5 + i] = th[i, j]
    for q in range(4):
        b["thetaf"][:, q * 25 : (q + 1) * 25, q * OUT : (q + 1) * OUT] = thf25[None]
    # thetaf/biasr are tiny; keep them per-core sharded (concat of copies)
    b["biasr"][...] = np.tile(bias.astype(np.float32), 4).reshape(P, 1)[None]
    return b


_RUNNER = None


def _make_runner(nc):
    """Build a cached jitted SPMD executor for the program (the stock
    run_bass_kernel_spmd re-traces and re-jits on every call, which costs
    seconds of host time per launch; this path jits once). Device output
    buffers are recycled across calls so the zeros upload happens once."""
    import jax
    import numpy as _np
    import concourse.mybir as mybir
    from concourse import bass2jax as b2j
    from jax.experimental.shard_map import shard_map
    from jax.sharding import Mesh, PartitionSpec

    b2j.install_neuronx_cc_hook()

    partition_name = nc.partition_id_tensor.name if nc.partition_id_tensor else None
    in_names, out_names, out_avals, zero_outs = [], [], [], []
    for alloc in nc.m.functions[0].allocations:
        if not isinstance(alloc, mybir.MemoryLocationSet):
            continue
        name = alloc.memorylocations[0].name
        if alloc.kind == "ExternalInput":
            if name != partition_name:
                in_names.append(name)
        elif alloc.kind == "ExternalOutput":
            shape = tuple(alloc.tensor_shape)
            dtype = mybir.dt.np(alloc.dtype)
            out_names.append(name)
            out_avals.append(jax.core.ShapedArray(shape, dtype))
            zero_outs.append(_np.zeros(shape, dtype))
    n_params = len(in_names)
    all_names = list(in_names) + list(out_names)
    if partition_name is not None:
        all_names.append(partition_name)
    donate = tuple(range(n_params, n_params + len(out_names)))

    def _body(*args):
        operands = list(args)
        if partition_name is not None:
            operands.append(b2j.partition_id_tensor())
        return tuple(
            b2j._bass_exec_p.bind(
                *operands,
                out_avals=tuple(out_avals),
                in_names=tuple(all_names),
                out_names=tuple(out_names),
                lowering_input_output_aliases=(),
                sim_require_finite=True,
                sim_require_nnan=True,
                nc=nc,
            )
        )

    devices = jax.devices()[:NCORES]
    mesh = Mesh(_np.asarray(devices), ("core",))
    nio = n_params + len(out_names)
    sharded = jax.jit(
        shard_map(
            _body,
            mesh=mesh,
            in_specs=(PartitionSpec("core"),) * nio,
            out_specs=(PartitionSpec("core"),) * len(out_names),
            check_rep=False,
        ),
        donate_argnums=donate,
        keep_unused=True,
    )

    out_cache = []
    # static inputs (content never changes between calls): upload once and
    # keep the device array resident
    static_names = {"ec"}
    dev_cache = {}

    def run(in_bufs):
        concat_in = []
        for name in in_names:
            a = in_bufs[name]
            if name in static_names:
                cached = dev_cache.get(name)
                if cached is None:
                    arr = a.reshape(-1, *a.shape[2:]) if a.ndim == 3 else a
                    cached = jax.device_put(
                        arr,
                        jax.sharding.NamedSharding(mesh, PartitionSpec("core")),
                    )
                    dev_cache[name] = cached
                concat_in.append(cached)
                continue
            if a.ndim == 3:
                a = a.reshape(-1, *a.shape[2:])
            concat_in.append(a)
        if out_cache:
            outs_in = list(out_cache)
            out_cache.clear()
        else:
            outs_in = [
                _np.zeros((NCORES * z.shape[0], *z.shape[1:]), z.dtype)
                for z in zero_outs
            ]
        out_arrs = sharded(*concat_in, *outs_in)
        res = {
            name: _np.asarray(out_arrs[i]).reshape(NCORES, *out_avals[i].shape)
            for i, name in enumerate(out_names)
        }
        out_cache.extend(out_arrs)
        return res

    return run


def kernel(x, Lr, Lc, theta, bias):
    global _BUILT, _RUNNER
    if _BUILT is None:
        _BUILT = _build_program()
    if _RUNNER is None:
        _RUNNER = _make_runner(_BUILT)
    in_bufs = _fill_inputs(
        np.asarray(x, dtype=np.float32),
        np.asarray(Lr, dtype=np.float32),
        np.asarray(Lc, dtype=np.float32),
        np.asarray(theta, dtype=np.float32),
        np.asarray(bias, dtype=np.float32),
    )
    res = _RUNNER(in_bufs)
    oc = res["outc"]  # (NCORES, OUT, ML, N) bf16
    out = np.empty((OUT, M, N), np.float32)
    for c in range(NCORES):
        out[:, c * ML : (c + 1) * ML, :] = oc[c]
    return out
